# revision 1
# baseline (speedup 1.0000x reference)
"""Conditional (per-row expert) linear layer for Trainium2, 8 NeuronCores.

Math: out[i] = W[c_i] @ x[i] + sum_c b[c]    (x: [B,D], W: [C,D,D], b: [C,D])

Strategy: expert-parallel. Core c handles exactly the rows with
condition_ids == c (gathered on host, padded to a common capacity so the
SPMD NEFF has static shapes). Each core runs one [n_cap, D] @ [D, D] GEMM
in float32r (fp32 bytes, FP22 multiply on the PE) plus a broadcast bias
add, then the host scatters rows back. This does 1/C of the reference's
dense FLOPs and reads only its own expert's weights.

Measured on the 8-core axon TRN2 (steady-state per-execution, rep-slope
method): ~40us; PE floor for the 144 N=512 matmuls is ~37.6us
(~261ns/MM regardless of dtype), DMA ~12.7MB/core/exec split across
HWDGE (W, half the outputs) and 4 SWDGE queues (x, other outputs).
Accuracy vs fp64 oracle: 1.25e-4 rms rel (float32r FP22 truncation).
"""

import sys
from contextlib import ExitStack

import numpy as np

try:
    import concourse.bass as bass  # noqa: F401
except ImportError:  # pragma: no cover
    sys.path.insert(0, "/opt/trn_rl_repo")

import jax
from jax.experimental.shard_map import shard_map
from jax.sharding import Mesh, PartitionSpec

import concourse.mybir as mybir
import concourse.tile as tile
from concourse import bacc
from concourse import bass2jax as _b2j

B, D, C = 8192, 1024, 8
P = 128  # partitions
KT = D // P  # k-tiles along the contraction dim
HALF = 512  # PSUM half-bank free size (fp32)

_cache: dict[int, "_Runner"] = {}


def _build(n_cap: int, reps: int = 1):
    """Per-core program: out[n, o] = xT.T @ WT + bias, n_cap x D output.

    reps > 1 repeats the whole body (including all DMAs) back-to-back for
    benchmarking: wall(T) - wall(1) isolates per-execution device time."""
    assert n_cap % 32 == 0
    row_tiles = [(s, min(P, n_cap - s)) for s in range(0, n_cap, P)]
    nc = bacc.Bacc("TRN2", target_bir_lowering=False, debug=False, num_devices=8, num_swdge_queues=4)
    xT = nc.dram_tensor("xT", [D, n_cap], mybir.dt.float32r, kind="ExternalInput").ap()
    WT = nc.dram_tensor("WT", [D, D], mybir.dt.float32r, kind="ExternalInput").ap()
    bias = nc.dram_tensor("bias", [1, D], mybir.dt.float32, kind="ExternalInput").ap()
    out = nc.dram_tensor("out", [n_cap, D], mybir.dt.float32, kind="ExternalOutput").ap()

    with tile.TileContext(nc) as tc, ExitStack() as ctx:
        w_pool = ctx.enter_context(tc.tile_pool(name="w", bufs=2))
        x_pool = ctx.enter_context(tc.tile_pool(name="x", bufs=2))
        b_pool = ctx.enter_context(tc.tile_pool(name="b", bufs=1))
        o_pool = ctx.enter_context(tc.tile_pool(name="o", bufs=1))
        ps_pool = ctx.enter_context(tc.tile_pool(name="ps", bufs=4, space="PSUM"))

        bias_row = b_pool.tile([1, D], mybir.dt.float32, name="bias_row", tag="bias_row")
        nc.sync.dma_start(bias_row[:], bias[:])
        bias_sb = b_pool.tile([P, D], mybir.dt.float32, name="bias_sb", tag="bias_sb")
        nc.gpsimd.partition_broadcast(bias_sb[:, :], bias_row[0:1, :])

        # Two k-groups with separate PSUM accumulations, combined on DVE.
        # A row-tile's group-A matmuls only need k0..k3 in SBUF, so the PE
        # starts long before the full input fill lands — the single-shot
        # fill (~9MB) overlaps the PE work instead of serializing it.
        k_groups = [range(0, KT // 2), range(KT // 2, KT)]
        xh = n_cap // 64 * 32  # x column split point (row dim), 32-aligned

        for _rep in range(reps):
            w_tiles, x_tiles = [], []
            for k in range(KT):
                wt = w_pool.tile([P, D], mybir.dt.float32r, name=f"wt{k}", tag=f"wt{k}")
                nc.sync.dma_start(wt[:, 0:HALF], WT[k * P : (k + 1) * P, 0:HALF])
                nc.sync.dma_start(wt[:, HALF:D], WT[k * P : (k + 1) * P, HALF:D])
                xt = x_pool.tile(
                    [P, n_cap], mybir.dt.float32r, name=f"xt{k}", tag=f"xt{k}"
                )
                nc.gpsimd.dma_start(xt[:, 0:xh], xT[k * P : (k + 1) * P, 0:xh])
                nc.gpsimd.dma_start(xt[:, xh:n_cap], xT[k * P : (k + 1) * P, xh:n_cap])
                w_tiles.append(wt)
                x_tiles.append(xt)

            o_tiles = {}
            for gi, ks in enumerate(k_groups):
                for start, size in row_tiles:
                    ps = ps_pool.tile([P, D], mybir.dt.float32, name="ps", tag="ps")
                    for k in ks:
                        for lo in (0, HALF):
                            nc.tensor.matmul(
                                ps[:size, lo : lo + HALF],
                                x_tiles[k][:, start : start + size],
                                w_tiles[k][:, lo : lo + HALF],
                                start=(k == ks[0]),
                                stop=(k == ks[-1]),
                                skip_group_check=True,
                            )
                    if gi == 0:
                        o_sb = o_pool.tile(
                            [P, D], mybir.dt.float32, name=f"o{start}", tag=f"o{start}"
                        )
                        o_tiles[start] = o_sb
                        nc.vector.tensor_add(
                            o_sb[:size, :], ps[:size, :], bias_sb[:size, :]
                        )
                    else:
                        o_sb = o_tiles[start]
                        nc.vector.tensor_add(o_sb[:size, :], o_sb[:size, :], ps[:size, :])
                    if gi == len(k_groups) - 1:
                        out_eng = nc.sync if (start // P) % 2 == 0 else nc.gpsimd
                        out_eng.dma_start(out[start : start + size, :], o_sb[:size, :])

    nc.compile()
    _check_noload_pairs(nc)
    return nc


def _check_noload_pairs(nc):
    """Every ldweights=False matmul must immediately follow (in PE stream
    order) a matmul with the identical stationary AP — otherwise the PE
    array would hold the wrong weights. Scheduling is deterministic at
    build time, so passing here guarantees correctness on device."""
    prev_mm = None
    for fn in nc.m.functions:
        for blk in fn.blocks:
            for inst in blk.instructions:
                if type(inst).__name__ != "InstMatmult":
                    continue
                if inst.ldweights is False:
                    assert prev_mm is not None, "no-load matmul with no predecessor"
                    same = str(prev_mm.ins[1]) == str(inst.ins[1])
                    assert same, (
                        f"no-load matmul stationary mismatch:\n"
                        f"prev: {prev_mm.ins[1]}\nthis: {inst.ins[1]}"
                    )
                prev_mm = inst


class _Runner:
    """Caches the compiled NEFF + jitted shard_map executable for one n_cap."""

    def __init__(self, n_cap: int, reps: int = 1):
        self.n_cap = n_cap
        self.nc = _build(n_cap, reps)
        _b2j.install_neuronx_cc_hook()

        assert self.nc.dbg_addr is None
        partition_name = (
            self.nc.partition_id_tensor.name if self.nc.partition_id_tensor else None
        )

        in_names, out_names, out_avals = [], [], []
        for alloc in self.nc.m.functions[0].allocations:
            if not isinstance(alloc, mybir.MemoryLocationSet):
                continue
            name = alloc.memorylocations[0].name
            if alloc.kind == "ExternalInput":
                if name != partition_name:
                    in_names.append(name)
            elif alloc.kind == "ExternalOutput":
                out_names.append(name)
                out_avals.append(
                    jax.core.ShapedArray(
                        tuple(alloc.tensor_shape), mybir.dt.np(alloc.dtype)
                    )
                )
        self.in_names = in_names
        self.out_names = out_names
        self.out_avals = out_avals
        self.n_params = len(in_names)
        self.n_outs = len(out_names)
        all_in_names = tuple(in_names + out_names)
        if partition_name is not None:
            all_in_names = all_in_names + (partition_name,)

        nc = self.nc

        def _bind(*args):
            operands = list(args)
            if partition_name is not None:
                operands.append(_b2j.partition_id_tensor())
            return tuple(
                _b2j._bass_exec_p.bind(
                    *operands,
                    out_avals=tuple(out_avals),
                    in_names=all_in_names,
                    out_names=tuple(out_names),
                    lowering_input_output_aliases=(),
                    sim_require_finite=True,
                    sim_require_nnan=True,
                    nc=nc,
                )
            )

        self._bind = _bind
        self.devices = jax.devices("neuron")[:C]
        self.mesh = Mesh(np.asarray(self.devices), ("core",))
        spec_in = (PartitionSpec("core"),) * (self.n_params + self.n_outs)
        spec_out = (PartitionSpec("core"),) * self.n_outs
        self._spec_in, self._spec_out = spec_in, spec_out
        self._exec = jax.jit(
            shard_map(
                _bind,
                mesh=self.mesh,
                in_specs=spec_in,
                out_specs=spec_out,
                check_rep=False,
            ),
            donate_argnums=tuple(range(self.n_params, self.n_params + self.n_outs)),
            keep_unused=True,
        )

    def make_exec_nodonate(self):
        """Jitted executable that does not donate its output-init operands,
        so pre-staged device args can be reused across timing reps."""
        return jax.jit(
            shard_map(
                self._bind,
                mesh=self.mesh,
                in_specs=self._spec_in,
                out_specs=self._spec_out,
                check_rep=False,
            ),
            keep_unused=True,
        )

    def concat_inputs(self, in_maps):
        return [
            np.concatenate([np.asarray(m[name]) for m in in_maps], axis=0)
            for name in self.in_names
        ]

    def zero_outs(self):
        return [
            np.zeros((C * a.shape[0], *a.shape[1:]), a.dtype) for a in self.out_avals
        ]

    def run(self, in_maps):
        out_arrs = self._exec(*self.concat_inputs(in_maps), *self.zero_outs())
        return [
            {
                name: np.asarray(out_arrs[i]).reshape(C, *self.out_avals[i].shape)[c]
                for i, name in enumerate(self.out_names)
            }
            for c in range(C)
        ]


def _get(n_cap: int, reps: int = 1) -> _Runner:
    key = (n_cap, reps)
    if key not in _cache:
        _cache[key] = _Runner(n_cap, reps)
    return _cache[key]


def _prep(x, condition_ids, W, b):
    x = np.ascontiguousarray(np.asarray(x, dtype=np.float32))
    cond = np.asarray(condition_ids).astype(np.int64)
    W = np.asarray(W, dtype=np.float32)
    b = np.asarray(b, dtype=np.float32)

    bias_tile = np.ascontiguousarray(b.sum(axis=0, dtype=np.float32)[None, :])

    rows = [np.nonzero(cond == c)[0] for c in range(C)]
    n_max = max(len(r) for r in rows)
    n_cap = max(32, -(-n_max // 32) * 32)

    in_maps = []
    for c in range(C):
        r = rows[c]
        xg = np.zeros((n_cap, D), np.float32)
        xg[: len(r)] = x[r]
        in_maps.append(
            {
                "xT": np.ascontiguousarray(xg.T),
                "WT": np.ascontiguousarray(W[c].T),
                "bias": bias_tile,
            }
        )
    return rows, n_cap, in_maps


def _run(x, condition_ids, W, b, trace=False):
    rows, n_cap, in_maps = _prep(x, condition_ids, W, b)
    runner = _get(n_cap)
    results = runner.run(in_maps)

    out = np.empty((B, D), np.float32)
    for c in range(C):
        r = rows[c]
        out[r] = results[c]["out"][: len(r)]
    return out, runner


def kernel(x, condition_ids, W, b):
    out, _ = _run(x, condition_ids, W, b)
    return out



# revision 7
# speedup vs baseline: 1.1295x; 1.1295x over previous
"""Conditional (per-row expert) linear layer for Trainium2, 8 NeuronCores.

Math: out[i] = W[c_i] @ x[i] + sum_c b[c]    (x: [B,D], W: [C,D,D], b: [C,D])

Strategy: expert-parallel. Core c handles exactly the rows with
condition_ids == c (gathered on host, padded to a common capacity n_cap so
the SPMD NEFF has static shapes). The core computes outT = WT.T-stationary
GEMM in bf16:

  outT[o, r] = sum_k WT[k, o] * xT[k, r]   (+ bias[o], per-partition)

with the [128,128] WT tile as the PE stationary operand (bf16 -> FWL fast
weight load, ~53ns) and xT chunks as the moving operand (1 col/cycle at
2.4GHz). Per (o,k) pair: one LDWEIGHTS + n_cap moving rows; 64 pairs total
per core -> PE floor ~= 64*(53 + n_cap/2.4)ns ~= 32us at n_cap=1056.
All DMA is bf16 (x, W, out) = ~6.3MB/core/exec, well under the PE span.
Bias is folded into the PSUM->SBUF evacuation (DVE tensor_scalar_add with
a per-partition [128,1] operand). Host does gather/transpose/scatter and
the bf16 casts (not counted in HW exec time).

Accuracy: bf16 inputs + fp32 PSUM accumulation + bf16 output ~= 2-3e-3
rms rel vs the f64 oracle (gate is 2e-2).
"""

import sys
from contextlib import ExitStack

import numpy as np

try:
    import concourse.bass as bass  # noqa: F401
except ImportError:  # pragma: no cover
    sys.path.insert(0, "/opt/trn_rl_repo")

import jax
from jax.experimental.shard_map import shard_map
from jax.sharding import Mesh, PartitionSpec

import ml_dtypes

import concourse.mybir as mybir
import concourse.tile as tile
from concourse import bacc
from concourse import bass2jax as _b2j

B, D, C = 8192, 1024, 8
P = 128  # partitions
KT = D // P  # k-tiles along the contraction dim
OT = D // P  # o-tiles along the output dim
BANK = 512  # PSUM bank free size (fp32)

BF16 = ml_dtypes.bfloat16

_cache: dict[tuple[int, int], "_Runner"] = {}


def _build(n_cap: int, reps: int = 1):
    """Per-core program: outT[o, r] = sum_k WT[k,o]*xT[k,r] + bias[o].

    reps > 1 repeats the whole body (including all DMAs) back-to-back for
    benchmarking: wall(T) - wall(1) isolates per-execution device time."""
    assert n_cap % 32 == 0
    chunks = [(lo, min(BANK, n_cap - lo)) for lo in range(0, n_cap, BANK)]
    nch = len(chunks)
    nc = bacc.Bacc("TRN2", target_bir_lowering=False, debug=False, num_devices=8, num_swdge_queues=4)
    xT = nc.dram_tensor("xT", [D, n_cap], mybir.dt.bfloat16, kind="ExternalInput").ap()
    WT = nc.dram_tensor("WT", [D, D], mybir.dt.bfloat16, kind="ExternalInput").ap()
    biasT = nc.dram_tensor("biasT", [P, OT], mybir.dt.float32, kind="ExternalInput").ap()
    outT = nc.dram_tensor("outT", [D, n_cap], mybir.dt.bfloat16, kind="ExternalOutput").ap()

    with tile.TileContext(nc) as tc, ExitStack() as ctx:
        w_pool = ctx.enter_context(tc.tile_pool(name="w", bufs=2))
        x_pool = ctx.enter_context(tc.tile_pool(name="x", bufs=2))
        b_pool = ctx.enter_context(tc.tile_pool(name="b", bufs=1))
        o_pool = ctx.enter_context(tc.tile_pool(name="o", bufs=3))
        ps_pool = ctx.enter_context(tc.tile_pool(name="ps", bufs=2, space="PSUM"))

        bias_sb = b_pool.tile([P, OT], mybir.dt.float32, name="bias_sb", tag="bias_sb")
        nc.sync.dma_start(bias_sb[:], biasT[:])

        for _rep in range(reps):
            w_tiles, x_tiles = [], []
            for k in range(KT):
                wt = w_pool.tile([P, D], mybir.dt.bfloat16, name=f"wt{k}", tag=f"wt{k}")
                nc.sync.dma_start(wt[:, :], WT[k * P : (k + 1) * P, :])
                xt = x_pool.tile(
                    [P, n_cap], mybir.dt.bfloat16, name=f"xt{k}", tag=f"xt{k}"
                )
                nc.gpsimd.dma_start(xt[:, :], xT[k * P : (k + 1) * P, :])
                w_tiles.append(wt)
                x_tiles.append(xt)

            for o in range(OT):
                ps = ps_pool.tile([P, BANK * nch], mybir.dt.float32, name="ps", tag="ps")
                for k in range(KT):
                    stat = w_tiles[k][:, o * P : (o + 1) * P]
                    for ci, (lo, sz) in enumerate(chunks):
                        nc.tensor.matmul(
                            ps[:, ci * BANK : ci * BANK + sz],
                            stat,
                            x_tiles[k][:, lo : lo + sz],
                            start=(k == 0),
                            stop=(k == KT - 1),
                            skip_group_check=True,
                        )
                o_sb = o_pool.tile(
                    [P, n_cap], mybir.dt.bfloat16, name=f"os{o}", tag="os"
                )
                nc.vector.tensor_scalar_add(
                    o_sb[:, :], ps[:, :n_cap], bias_sb[:, o : o + 1]
                )
                out_eng = nc.scalar if o % 2 == 0 else nc.sync
                out_eng.dma_start(outT[o * P : (o + 1) * P, :], o_sb[:, :])

    _dedupe_ldweights(nc)
    nc.compile()
    _check_pe_stream(nc, reps, nch)
    return nc


def _dedupe_ldweights(nc):
    """The Tile legalizer inserts one InstLdweights before EVERY matmul with
    a non-f32 moving operand, even when consecutive matmuls share the
    stationary tile. Redundant loads cost ~50-110ns of PE time each. Drop an
    LDW when the immediately-preceding PE-stream weight load had the
    identical AP (only no-load matmuls between, which keep the PE array
    state). The dropped LDWs carry only a duplicate sync dep on the
    weight-tile DMA (same as the kept LDW) and nothing depends on them —
    verified by construction below (assert)."""
    for fn in nc.m.functions:
        for blk in fn.blocks:
            removed_names = set()
            new_insts = []
            last_ldw_ap = None
            for inst in blk.instructions:
                nm = type(inst).__name__
                if nm == "InstLdweights":
                    ap = str(inst.ins[0])
                    if ap == last_ldw_ap:
                        removed_names.add(inst.name)
                        continue
                    last_ldw_ap = ap
                elif nm == "InstMatmult":
                    pass  # no-load matmul keeps the array's weight state
                elif inst.engine == mybir.EngineType.PE:
                    last_ldw_ap = None
                new_insts.append(inst)
            if removed_names:
                for inst in new_insts:
                    for dep, _info in inst.dependency_edges():
                        assert dep not in removed_names, (inst.name, dep)
                blk.instructions[:] = new_insts


def _check_pe_stream(nc, reps, nch):
    """Every matmul must run with the correct weights resident: in PE stream
    order, the most recent InstLdweights must carry the matmul's stationary
    AP (self-loading matmuls track their own). Scheduling is deterministic
    at build time, so passing here guarantees correctness on device."""
    n_loads = 0
    n_mm = 0
    for fn in nc.m.functions:
        for blk in fn.blocks:
            loaded_ap = None
            for inst in blk.instructions:
                nm = type(inst).__name__
                if nm == "InstLdweights":
                    loaded_ap = str(inst.ins[0])
                    n_loads += 1
                elif nm == "InstMatmult":
                    n_mm += 1
                    if inst.ldweights is False:
                        assert loaded_ap is not None, "no-load matmul, no LDW"
                        assert str(inst.ins[1]) == loaded_ap, (
                            f"no-load matmul stationary mismatch:\n"
                            f"loaded: {loaded_ap}\nthis: {inst.ins[1]}"
                        )
                    else:
                        loaded_ap = str(inst.ins[1])
    assert n_mm == reps * KT * OT * nch, (n_mm, reps)
    assert n_loads <= reps * (KT * OT + 16), (n_loads, n_mm, reps)


class _Runner:
    """Caches the compiled NEFF + jitted shard_map executable for one n_cap."""

    def __init__(self, n_cap: int, reps: int = 1):
        self.n_cap = n_cap
        self.nc = _build(n_cap, reps)
        _b2j.install_neuronx_cc_hook()

        assert self.nc.dbg_addr is None
        partition_name = (
            self.nc.partition_id_tensor.name if self.nc.partition_id_tensor else None
        )

        in_names, out_names, out_avals = [], [], []
        for alloc in self.nc.m.functions[0].allocations:
            if not isinstance(alloc, mybir.MemoryLocationSet):
                continue
            name = alloc.memorylocations[0].name
            if alloc.kind == "ExternalInput":
                if name != partition_name:
                    in_names.append(name)
            elif alloc.kind == "ExternalOutput":
                out_names.append(name)
                out_avals.append(
                    jax.core.ShapedArray(
                        tuple(alloc.tensor_shape), mybir.dt.np(alloc.dtype)
                    )
                )
        self.in_names = in_names
        self.out_names = out_names
        self.out_avals = out_avals
        self.n_params = len(in_names)
        self.n_outs = len(out_names)
        all_in_names = tuple(in_names + out_names)
        if partition_name is not None:
            all_in_names = all_in_names + (partition_name,)

        nc = self.nc

        def _bind(*args):
            operands = list(args)
            if partition_name is not None:
                operands.append(_b2j.partition_id_tensor())
            return tuple(
                _b2j._bass_exec_p.bind(
                    *operands,
                    out_avals=tuple(out_avals),
                    in_names=all_in_names,
                    out_names=tuple(out_names),
                    lowering_input_output_aliases=(),
                    sim_require_finite=True,
                    sim_require_nnan=True,
                    nc=nc,
                )
            )

        self._bind = _bind
        self.devices = jax.devices("neuron")[:C]
        self.mesh = Mesh(np.asarray(self.devices), ("core",))
        spec_in = (PartitionSpec("core"),) * (self.n_params + self.n_outs)
        spec_out = (PartitionSpec("core"),) * self.n_outs
        self._spec_in, self._spec_out = spec_in, spec_out
        self._exec = jax.jit(
            shard_map(
                _bind,
                mesh=self.mesh,
                in_specs=spec_in,
                out_specs=spec_out,
                check_rep=False,
            ),
            donate_argnums=tuple(range(self.n_params, self.n_params + self.n_outs)),
            keep_unused=True,
        )

    def make_exec_nodonate(self):
        """Jitted executable that does not donate its output-init operands,
        so pre-staged device args can be reused across timing reps."""
        return jax.jit(
            shard_map(
                self._bind,
                mesh=self.mesh,
                in_specs=self._spec_in,
                out_specs=self._spec_out,
                check_rep=False,
            ),
            keep_unused=True,
        )

    def concat_inputs(self, in_maps):
        return [
            np.concatenate([np.asarray(m[name]) for m in in_maps], axis=0)
            for name in self.in_names
        ]

    def zero_outs(self):
        return [
            np.zeros((C * a.shape[0], *a.shape[1:]), a.dtype) for a in self.out_avals
        ]

    def run(self, in_maps):
        out_arrs = self._exec(*self.concat_inputs(in_maps), *self.zero_outs())
        return [
            {
                name: np.asarray(out_arrs[i]).reshape(C, *self.out_avals[i].shape)[c]
                for i, name in enumerate(self.out_names)
            }
            for c in range(C)
        ]


def _get(n_cap: int, reps: int = 1) -> _Runner:
    key = (n_cap, reps)
    if key not in _cache:
        _cache[key] = _Runner(n_cap, reps)
    return _cache[key]


def _prep(x, condition_ids, W, b):
    x = np.asarray(x, dtype=np.float32)
    cond = np.asarray(condition_ids).astype(np.int64)
    W = np.asarray(W, dtype=np.float32)
    b = np.asarray(b, dtype=np.float32)

    bias_sum = b.sum(axis=0, dtype=np.float32)  # [D]
    biasT = np.ascontiguousarray(bias_sum.reshape(OT, P).T)  # [P, OT]

    rows = [np.nonzero(cond == c)[0] for c in range(C)]
    n_max = max(len(r) for r in rows)
    n_cap = max(32, -(-n_max // 32) * 32)

    in_maps = []
    for c in range(C):
        r = rows[c]
        xTg = np.zeros((D, n_cap), BF16)
        xTg[:, : len(r)] = x[r].T
        in_maps.append(
            {
                "xT": xTg,
                "WT": np.ascontiguousarray(W[c].T).astype(BF16),
                "biasT": biasT,
            }
        )
    return rows, n_cap, in_maps


def _run(x, condition_ids, W, b, trace=False):
    rows, n_cap, in_maps = _prep(x, condition_ids, W, b)
    runner = _get(n_cap)
    results = runner.run(in_maps)

    out = np.empty((B, D), np.float32)
    for c in range(C):
        r = rows[c]
        out[r] = results[c]["outT"][:, : len(r)].T.astype(np.float32)
    return out, runner


def kernel(x, condition_ids, W, b):
    out, _ = _run(x, condition_ids, W, b)
    return out


# revision 15
# speedup vs baseline: 1.1755x; 1.0407x over previous
"""Conditional (per-row expert) linear layer for Trainium2, 8 NeuronCores.

Math: out[i] = W[c_i] @ x[i] + sum_c b[c]    (x: [B,D], W: [C,D,D], b: [C,D])

Strategy: expert-parallel. Core c handles exactly the rows with
condition_ids == c (gathered on host, padded to a common capacity n_cap so
the SPMD NEFF has static shapes). The core computes outT = WT.T-stationary
GEMM in bf16:

  outT[o, r] = sum_k WT[k, o] * xT[k, r]   (+ bias[o], per-partition)

with the [128,128] WT tile as the PE stationary operand (bf16 -> FWL fast
weight load, ~53ns) and xT chunks as the moving operand (1 col/cycle at
2.4GHz). Per (o,k) pair: one LDWEIGHTS + n_cap moving rows; 64 pairs total
per core -> PE floor ~= 64*(53 + n_cap/2.4)ns ~= 32us at n_cap=1056.
All DMA is bf16 (x, W, out) = ~6.3MB/core/exec, well under the PE span.
Bias is folded into the PSUM->SBUF evacuation (DVE tensor_scalar_add with
a per-partition [128,1] operand). Host does gather/transpose/scatter and
the bf16 casts (not counted in HW exec time).

Accuracy: bf16 inputs + fp32 PSUM accumulation + bf16 output ~= 2-3e-3
rms rel vs the f64 oracle (gate is 2e-2).
"""

import sys
from contextlib import ExitStack

import numpy as np

try:
    import concourse.bass as bass  # noqa: F401
except ImportError:  # pragma: no cover
    sys.path.insert(0, "/opt/trn_rl_repo")

import jax
from jax.experimental.shard_map import shard_map
from jax.sharding import Mesh, PartitionSpec

import ml_dtypes

import concourse.mybir as mybir
import concourse.tile as tile
from concourse import bacc
from concourse import bass2jax as _b2j

B, D, C = 8192, 1024, 8
P = 128  # partitions
KT = D // P  # k-tiles along the contraction dim
OT = D // P  # o-tiles along the output dim
BANK = 512  # PSUM bank free size (fp32)

BF16 = ml_dtypes.bfloat16

_cache: dict[tuple[int, int], "_Runner"] = {}


def _chunks(n_cap: int):
    """Split n_cap moving rows into equal 32-aligned chunks, each <= 512
    (PSUM bank) and kept >= 256 where possible (float32r moving streams at
    1 cy/row only when the free dim is >= 256)."""
    nch = -(-n_cap // BANK)
    base = (n_cap // nch) // 32 * 32
    sizes = [base] * nch
    rem = n_cap - base * nch
    assert rem % 32 == 0
    for i in range(rem // 32):
        sizes[i % nch] += 32
    out, lo = [], 0
    for sz in sizes:
        out.append((lo, sz))
        lo += sz
    return out


def _build(n_cap: int, reps: int = 1):
    """Per-core program: outT[o, r] = sum_k WT[k,o]*xT[k,r] + bias[o].

    All-bf16 matmuls: the Tile legalizer splits each one into a standalone
    InstLdweights + no-load matmul; _dedupe_ldweights then removes the
    redundant loads so each [128,128] WT stationary loads once per (o,k)
    pair (~107ns) and the row chunks stream back-to-back at 1 col/cycle.
    A nosync dependency chain pins the PE stream to program order so the
    dedupe finds every redundant load (the scheduler otherwise interleaves
    o-groups at boundaries); _check_pe_stream verifies the weight-state
    invariant post-compile.

    reps > 1 repeats the whole body (including all DMAs) back-to-back for
    benchmarking: wall(T) - wall(1) isolates per-execution device time."""
    assert n_cap % 32 == 0
    chunks = _chunks(n_cap)
    nch = len(chunks)
    nc = bacc.Bacc("TRN2", target_bir_lowering=False, debug=False, num_devices=8, num_swdge_queues=4)
    xT = nc.dram_tensor("xT", [D, n_cap], mybir.dt.bfloat16, kind="ExternalInput").ap()
    WT = nc.dram_tensor("WT", [D, D], mybir.dt.bfloat16, kind="ExternalInput").ap()
    biasT = nc.dram_tensor("biasT", [P, OT], mybir.dt.float32, kind="ExternalInput").ap()
    outT = nc.dram_tensor("outT", [D, n_cap], mybir.dt.bfloat16, kind="ExternalOutput").ap()

    with tile.TileContext(nc) as tc, ExitStack() as ctx:
        w_pool = ctx.enter_context(tc.tile_pool(name="w", bufs=2))
        x_pool = ctx.enter_context(tc.tile_pool(name="x", bufs=2))
        b_pool = ctx.enter_context(tc.tile_pool(name="b", bufs=1))
        o_pool = ctx.enter_context(tc.tile_pool(name="o", bufs=3))
        ps_pool = ctx.enter_context(tc.tile_pool(name="ps", bufs=2, space="PSUM"))

        bias_sb = b_pool.tile([P, OT], mybir.dt.float32, name="bias_sb", tag="bias_sb")
        nc.sync.dma_start(bias_sb[:], biasT[:])

        prev_mm = None
        for _rep in range(reps):
            w_tiles, x_tiles = [], []
            for k in range(KT):
                wt = w_pool.tile([P, D], mybir.dt.bfloat16, name=f"wt{k}", tag=f"wt{k}")
                nc.sync.dma_start(wt[:, :], WT[k * P : (k + 1) * P, :])
                xt = x_pool.tile(
                    [P, n_cap], mybir.dt.bfloat16, name=f"xt{k}", tag=f"xt{k}"
                )
                nc.gpsimd.dma_start(xt[:, :], xT[k * P : (k + 1) * P, :])
                w_tiles.append(wt)
                x_tiles.append(xt)

            for o in range(OT):
                ps = ps_pool.tile([P, BANK * nch], mybir.dt.float32, name="ps", tag="ps")
                for k in range(KT):
                    stat = w_tiles[k][:, o * P : (o + 1) * P]
                    for ci, (lo, sz) in enumerate(chunks):
                        mm = nc.tensor.matmul(
                            ps[:, ci * BANK : ci * BANK + sz],
                            stat,
                            x_tiles[k][:, lo : lo + sz],
                            start=(k == 0),
                            stop=(k == KT - 1),
                            skip_group_check=True,
                        )
                        if prev_mm is not None:
                            mm.ins.add_dependency(
                                prev_mm, mybir.DependencyInfo.NO_SYNC_ONLY
                            )
                        prev_mm = mm.ins.name
                o_sb = o_pool.tile(
                    [P, n_cap], mybir.dt.bfloat16, name=f"os{o}", tag="os"
                )
                for ci, (lo, sz) in enumerate(chunks):
                    nc.vector.tensor_scalar_add(
                        o_sb[:, lo : lo + sz],
                        ps[:, ci * BANK : ci * BANK + sz],
                        bias_sb[:, o : o + 1],
                    )
                out_eng = nc.scalar if o % 2 == 0 else nc.sync
                out_eng.dma_start(outT[o * P : (o + 1) * P, :], o_sb[:, :])

    _dedupe_ldweights(nc)
    nc.compile()
    _check_pe_stream(nc, reps, nch)
    return nc


def _dedupe_ldweights(nc):
    """The Tile legalizer inserts one InstLdweights before EVERY matmul with
    a non-f32 moving operand, even when consecutive matmuls share the
    stationary tile. Redundant loads cost ~50-110ns of PE time each. Drop an
    LDW when the immediately-preceding PE-stream weight load had the
    identical AP (only no-load matmuls between, which keep the PE array
    state). The dropped LDWs carry only a duplicate sync dep on the
    weight-tile DMA (same as the kept LDW) and nothing depends on them —
    verified by construction below (assert)."""
    for fn in nc.m.functions:
        for blk in fn.blocks:
            removed_names = set()
            new_insts = []
            last_ldw_ap = None
            for inst in blk.instructions:
                nm = type(inst).__name__
                if nm == "InstLdweights":
                    ap = str(inst.ins[0])
                    if ap == last_ldw_ap:
                        removed_names.add(inst.name)
                        continue
                    last_ldw_ap = ap
                elif nm == "InstMatmult":
                    pass  # no-load matmul keeps the array's weight state
                elif inst.engine == mybir.EngineType.PE:
                    last_ldw_ap = None
                new_insts.append(inst)
            if removed_names:
                for inst in new_insts:
                    for dep, _info in inst.dependency_edges():
                        assert dep not in removed_names, (inst.name, dep)
                blk.instructions[:] = new_insts


def _check_pe_stream(nc, reps, nch):
    """Every matmul must run with the correct weights resident: in PE stream
    order, the most recent weight load (standalone InstLdweights or a
    self-loading matmul) must carry this matmul's stationary AP. Scheduling
    is deterministic at build time, so passing here guarantees correctness
    on device."""
    n_loads = 0
    n_mm = 0
    for fn in nc.m.functions:
        for blk in fn.blocks:
            loaded_ap = None
            for inst in blk.instructions:
                nm = type(inst).__name__
                if nm == "InstLdweights":
                    loaded_ap = str(inst.ins[0])
                    n_loads += 1
                elif nm == "InstMatmult":
                    n_mm += 1
                    if inst.ldweights is False:
                        assert loaded_ap is not None, "no-load matmul, no LDW"
                        assert str(inst.ins[1]) == loaded_ap, (
                            f"no-load matmul stationary mismatch:\n"
                            f"loaded: {loaded_ap}\nthis: {inst.ins[1]}"
                        )
                    else:
                        loaded_ap = str(inst.ins[1])
                        n_loads += 1
    assert n_mm == reps * KT * OT * nch, (n_mm, reps, nch)
    assert n_loads <= reps * (KT * OT + 16), (n_loads, n_mm, reps)


class _Runner:
    """Caches the compiled NEFF + jitted shard_map executable for one n_cap."""

    def __init__(self, n_cap: int, reps: int = 1):
        self.n_cap = n_cap
        self.nc = _build(n_cap, reps)
        _b2j.install_neuronx_cc_hook()

        assert self.nc.dbg_addr is None
        partition_name = (
            self.nc.partition_id_tensor.name if self.nc.partition_id_tensor else None
        )

        in_names, out_names, out_avals = [], [], []
        for alloc in self.nc.m.functions[0].allocations:
            if not isinstance(alloc, mybir.MemoryLocationSet):
                continue
            name = alloc.memorylocations[0].name
            if alloc.kind == "ExternalInput":
                if name != partition_name:
                    in_names.append(name)
            elif alloc.kind == "ExternalOutput":
                out_names.append(name)
                out_avals.append(
                    jax.core.ShapedArray(
                        tuple(alloc.tensor_shape), mybir.dt.np(alloc.dtype)
                    )
                )
        self.in_names = in_names
        self.out_names = out_names
        self.out_avals = out_avals
        self.n_params = len(in_names)
        self.n_outs = len(out_names)
        all_in_names = tuple(in_names + out_names)
        if partition_name is not None:
            all_in_names = all_in_names + (partition_name,)

        nc = self.nc

        def _bind(*args):
            operands = list(args)
            if partition_name is not None:
                operands.append(_b2j.partition_id_tensor())
            return tuple(
                _b2j._bass_exec_p.bind(
                    *operands,
                    out_avals=tuple(out_avals),
                    in_names=all_in_names,
                    out_names=tuple(out_names),
                    lowering_input_output_aliases=(),
                    sim_require_finite=True,
                    sim_require_nnan=True,
                    nc=nc,
                )
            )

        self._bind = _bind
        self.devices = jax.devices("neuron")[:C]
        self.mesh = Mesh(np.asarray(self.devices), ("core",))
        spec_in = (PartitionSpec("core"),) * (self.n_params + self.n_outs)
        spec_out = (PartitionSpec("core"),) * self.n_outs
        self._spec_in, self._spec_out = spec_in, spec_out
        self._exec = jax.jit(
            shard_map(
                _bind,
                mesh=self.mesh,
                in_specs=spec_in,
                out_specs=spec_out,
                check_rep=False,
            ),
            donate_argnums=tuple(range(self.n_params, self.n_params + self.n_outs)),
            keep_unused=True,
        )

    def make_exec_nodonate(self):
        """Jitted executable that does not donate its output-init operands,
        so pre-staged device args can be reused across timing reps."""
        return jax.jit(
            shard_map(
                self._bind,
                mesh=self.mesh,
                in_specs=self._spec_in,
                out_specs=self._spec_out,
                check_rep=False,
            ),
            keep_unused=True,
        )

    def concat_inputs(self, in_maps):
        return [
            np.concatenate([np.asarray(m[name]) for m in in_maps], axis=0)
            for name in self.in_names
        ]

    def zero_outs(self):
        return [
            np.zeros((C * a.shape[0], *a.shape[1:]), a.dtype) for a in self.out_avals
        ]

    def run(self, in_maps):
        out_arrs = self._exec(*self.concat_inputs(in_maps), *self.zero_outs())
        return [
            {
                name: np.asarray(out_arrs[i]).reshape(C, *self.out_avals[i].shape)[c]
                for i, name in enumerate(self.out_names)
            }
            for c in range(C)
        ]


def _get(n_cap: int, reps: int = 1) -> _Runner:
    key = (n_cap, reps)
    if key not in _cache:
        _cache[key] = _Runner(n_cap, reps)
    return _cache[key]


def _prep(x, condition_ids, W, b):
    x = np.asarray(x, dtype=np.float32)
    cond = np.asarray(condition_ids).astype(np.int64)
    W = np.asarray(W, dtype=np.float32)
    b = np.asarray(b, dtype=np.float32)

    bias_sum = b.sum(axis=0, dtype=np.float32)  # [D]
    biasT = np.ascontiguousarray(bias_sum.reshape(OT, P).T)  # [P, OT]

    rows = [np.nonzero(cond == c)[0] for c in range(C)]
    n_max = max(len(r) for r in rows)
    n_cap = max(32, -(-n_max // 32) * 32)

    in_maps = []
    for c in range(C):
        r = rows[c]
        xTg = np.zeros((D, n_cap), BF16)
        xTg[:, : len(r)] = x[r].T
        in_maps.append(
            {
                "xT": xTg,
                "WT": np.ascontiguousarray(W[c].T).astype(BF16),
                "biasT": biasT,
            }
        )
    return rows, n_cap, in_maps


def _run(x, condition_ids, W, b, trace=False):
    rows, n_cap, in_maps = _prep(x, condition_ids, W, b)
    runner = _get(n_cap)
    results = runner.run(in_maps)

    out = np.empty((B, D), np.float32)
    for c in range(C):
        r = rows[c]
        out[r] = results[c]["outT"][:, : len(r)].T.astype(np.float32)
    return out, runner


def kernel(x, condition_ids, W, b):
    out, _ = _run(x, condition_ids, W, b)
    return out
